# revision 2
# baseline (speedup 1.0000x reference)
"""LoRA layer (x @ W.T + (x@A)@B + bias) on 8 trn2 NeuronCores.

Data-parallel: core b computes batch b's (2048, 4096) output slice.
The low-rank path is folded on the host (W_eff.T = W.T + A@B, a rank-16
update, 0.4% of the FLOPs), so the device kernel is a pure GEMM + bias.
GEMM operands are bf16: the PE streams 2 bf16 moving elements/cycle vs
1 for fp32(r), doubling matmul throughput; rel-err stays ~1e-3, well
under the 2e-2 gate.

Per-core schedule: x^T (bf16, 128 KB/partition) is fully resident in
SBUF; W_eff^T streams through once (32 MiB). For each 512-wide output
column group, 32 K-tiles of W are fetched, then two 1024-token
super-chunks accumulate 32 K=128 steps into 8 PSUM banks (4 col-blocks
x 2 token-chunks). Bias is added during the PSUM->SBUF eviction
(per-partition tensor_scalar add) and the output is DMA'd transposed
([DOUT, SEQ] bf16) and re-transposed/upcast on the host.
"""
import contextlib

import ml_dtypes
import numpy as np

import concourse.mybir as mybir
import concourse.tile as tile
from concourse import bacc
from concourse.bass_utils import run_bass_kernel_spmd

BATCH, SEQ, DIN, DOUT, RANK = 8, 2048, 4096, 4096, 16
N_CORES = 8

KT = DIN // 128          # 32 contraction tiles
OG = DOUT // 512         # 8 output column groups
SC = SEQ // 1024         # 2 token super-chunks
BF16 = mybir.dt.bfloat16
F32 = mybir.dt.float32
NP_BF16 = ml_dtypes.bfloat16

_nc_cache = {}


def build(reps=1):
    nc = bacc.Bacc("TRN2", target_bir_lowering=False, debug=False)
    xT = nc.dram_tensor("xT", [DIN, SEQ], BF16, kind="ExternalInput")
    wT = nc.dram_tensor("wT", [DIN, DOUT], BF16, kind="ExternalInput")
    biasT = nc.dram_tensor("biasT", [128, DOUT // 128], F32, kind="ExternalInput")
    outT = nc.dram_tensor("outT", [DOUT, SEQ], BF16, kind="ExternalOutput")

    with tile.TileContext(nc) as tc:
        with (
            tc.tile_pool(name="xblk", bufs=KT) as xpool,
            tc.tile_pool(name="wt", bufs=KT + 8) as wpool,
            tc.tile_pool(name="bias", bufs=1) as bpool,
            tc.tile_pool(name="outp", bufs=8) as opool,
            tc.tile_pool(name="psum", bufs=8, space="PSUM") as ppool,
        ):
            bias_sb = bpool.tile([128, DOUT // 128], F32, tag="bias")
            nc.sync.dma_start(bias_sb[:], biasT[:, :])

            rep_ctx = tc.For_i(0, reps, 1) if reps > 1 else contextlib.nullcontext()
            with rep_ctx:
                xtiles = [None] * KT
                for og in range(OG):
                    og0 = og * 512
                    wts = []
                    for k in range(KT):
                        wt_t = wpool.tile([128, 512], BF16, name="w", tag="w")
                        nc.sync.dma_start(
                            wt_t[:], wT[k * 128:(k + 1) * 128, og0:og0 + 512])
                        wts.append(wt_t)
                    for sc in range(SC):
                        s0 = sc * 1024
                        psums = [ppool.tile([128, 512], F32, name="ps", tag="ps")
                                 for _ in range(8)]
                        for k in range(KT):
                            if xtiles[k] is None:
                                xt = xpool.tile([128, SEQ], BF16, name="x", tag="x")
                                nc.sync.dma_start(
                                    xt[:], xT[k * 128:(k + 1) * 128, :])
                                xtiles[k] = xt
                            for oi in range(4):
                                for mc in range(2):
                                    nc.tensor.matmul(
                                        psums[oi * 2 + mc][:],
                                        wts[k][:, oi * 128:(oi + 1) * 128],
                                        xtiles[k][:, s0 + mc * 512:s0 + (mc + 1) * 512],
                                        start=(k == 0), stop=(k == KT - 1))
                        for oi in range(4):
                            for mc in range(2):
                                ot = opool.tile([128, 512], BF16, name="o", tag="o")
                                nc.vector.tensor_scalar_add(
                                    ot[:], psums[oi * 2 + mc][:],
                                    bias_sb[:, og * 4 + oi:og * 4 + oi + 1])
                                nc.sync.dma_start(
                                    outT[og0 + oi * 128:og0 + (oi + 1) * 128,
                                         s0 + mc * 512:s0 + (mc + 1) * 512],
                                    ot[:])
    nc.compile()
    return nc


def prepare_inputs(x, A, B, weight, bias):
    x = np.asarray(x, dtype=np.float32)
    A = np.asarray(A, dtype=np.float32)
    B = np.asarray(B, dtype=np.float32)
    weight = np.asarray(weight, dtype=np.float32)
    bias = np.asarray(bias, dtype=np.float32)

    wT_eff = (weight.T + A @ B).astype(NP_BF16)              # [DIN, DOUT]
    biasT = np.ascontiguousarray(
        bias.reshape(DOUT // 128, 128).T)                    # [128, 32]

    in_maps = []
    for b in range(N_CORES):
        xTb = np.ascontiguousarray(x[b].astype(NP_BF16).T)   # [DIN, SEQ]
        in_maps.append({"xT": xTb, "wT": wT_eff, "biasT": biasT})
    return in_maps


def assemble(results):
    return np.stack(
        [np.ascontiguousarray(r["outT"].astype(np.float32).T)
         for r in results], axis=0)


def kernel(x, A, B, weight, bias):
    if 1 not in _nc_cache:
        _nc_cache[1] = build(reps=1)
    nc = _nc_cache[1]
    in_maps = prepare_inputs(x, A, B, weight, bias)
    res = run_bass_kernel_spmd(nc, in_maps, core_ids=list(range(N_CORES)))
    last_result.clear()
    last_result.append(res)
    return assemble(res.results)


last_result = []
